# revision 4
# baseline (speedup 1.0000x reference)
"""Segment mean-pool (ContextEncoder) Bass kernel for Trainium2, 8 NeuronCores.

Problem: hidden_states [16, 4096, 1024] f32, output_mask [16, 4096] int
(non-decreasing segment ids per row in [0, 512), -1 = dropped token).
Output [16*512, 1024] f32: mean of tokens sharing (batch, segment id),
zeros for empty segments.

Strategy: data-parallel over batch, 2 rows per core. Per 128-token K-tile,
build a one-hot [tokens x 512 segments] matrix on the vector engine
(iota vs per-partition segment id, is_equal), then accumulate
one_hot.T @ x on the tensor engine (fp32r) into PSUM, one [128 seg x 1024]
region per 128-segment M-tile. Because ids are sorted, each K-tile only
touches 1-2 M-tiles; the (k -> M-tiles) map is computed on the host from
the actual masks (the program is compiled per input batch) so the matmul
count stays near the minimum while remaining exact for any mask content.
Mean = PSUM * (1/count) on the way out, with counts from a host bincount.
"""

import numpy as np

import concourse.bass as bass  # noqa: F401  (registers bass_rust)
import concourse.mybir as mybir
import concourse.tile as tile
from concourse import bacc
from concourse.bass_utils import run_bass_kernel_spmd

B, S, H = 16, 4096, 1024
NSEG = 512
NCORES = 8
RPC = B // NCORES          # rows (batch examples) per core
P = 128                    # partitions
KT = S // P                # 32 K-tiles of 128 tokens
MT = NSEG // P             # 4 M-tiles of 128 segments
NH = H // 512              # matmul free-dim chunks (PSUM bank = 512 f32)

F32 = mybir.dt.float32
F32R = mybir.dt.float32r   # full-rate fp32 matmul mode on TRN2

# Number of SBUF buffers for data tiles (DMA prefetch depth)
DATA_BUFS = 8
OH_BUFS = 6
OSB_BUFS = 4


def _build_program(klists):
    """klists[r][m] -> sorted list of K-tile indices whose token ids (in any
    row assigned to program slot r) overlap segment M-tile m. Must be
    non-empty for every (r, m)."""
    nc = bacc.Bacc("TRN2", target_bir_lowering=False, debug=False)
    x = nc.dram_tensor("x", [RPC, S, H], F32R, kind="ExternalInput")
    maskp = nc.dram_tensor("maskp", [RPC, P, KT], F32, kind="ExternalInput")
    invc = nc.dram_tensor("invc", [RPC, P, MT], F32, kind="ExternalInput")
    out = nc.dram_tensor("out", [RPC, NSEG, H], F32, kind="ExternalOutput")

    with tile.TileContext(nc) as tc:
        with tc.tile_pool(name="const", bufs=1) as cpool, \
             tc.tile_pool(name="data", bufs=DATA_BUFS) as dpool, \
             tc.tile_pool(name="oh", bufs=OH_BUFS) as opool, \
             tc.tile_pool(name="osb", bufs=OSB_BUFS) as spool, \
             tc.tile_pool(name="ps", bufs=MT, space="PSUM") as pspool:
            iota_t = cpool.tile([P, NSEG], F32, tag="iota")
            nc.gpsimd.iota(iota_t[:], [[1, NSEG]], channel_multiplier=0,
                           allow_small_or_imprecise_dtypes=True)
            for r in range(RPC):
                mask_sb = cpool.tile([P, KT], F32, tag=f"mask{r}")
                nc.sync.dma_start(out=mask_sb[:], in_=maskp[r])
                invc_sb = cpool.tile([P, MT], F32, tag=f"invc{r}")
                nc.sync.dma_start(out=invc_sb[:], in_=invc[r])

                k_to_ms = {}
                for m in range(MT):
                    for k in klists[r][m]:
                        k_to_ms.setdefault(k, []).append(m)
                firsts = {m: klists[r][m][0] for m in range(MT)}
                lasts = {m: klists[r][m][-1] for m in range(MT)}

                psum = [pspool.tile([P, H], F32, tag="ps", name=f"psum_r{r}m{m}")
                        for m in range(MT)]

                for k in sorted(k_to_ms):
                    data_t = dpool.tile([P, H], F32R, tag="data")
                    nc.sync.dma_start(out=data_t[:], in_=x[r, k * P:(k + 1) * P, :])
                    oh = opool.tile([P, NSEG], F32R, tag="oh")
                    nc.vector.tensor_scalar(
                        out=oh[:], in0=iota_t[:], scalar1=mask_sb[:, k:k + 1],
                        scalar2=None, op0=mybir.AluOpType.is_equal)
                    for m in k_to_ms[k]:
                        for n in range(NH):
                            nc.tensor.matmul(
                                out=psum[m][:, n * 512:(n + 1) * 512],
                                lhsT=oh[:, m * P:(m + 1) * P],
                                rhs=data_t[:, n * 512:(n + 1) * 512],
                                start=(k == firsts[m]), stop=(k == lasts[m]))

                for m in range(MT):
                    osb = spool.tile([P, H], F32, tag="osb")
                    nc.vector.tensor_scalar_mul(osb[:], psum[m][:], invc_sb[:, m:m + 1])
                    nc.sync.dma_start(out=out[r, m * P:(m + 1) * P, :], in_=osb[:])
    nc.compile()
    return nc


def _prep(hidden_states, output_mask):
    hs = np.ascontiguousarray(np.asarray(hidden_states, dtype=np.float32))
    assert hs.shape == (B, S, H), hs.shape
    mask = np.asarray(output_mask).astype(np.int64)
    assert mask.shape == (B, S), mask.shape

    valid = mask >= 0
    # per-(row, K-tile) id range over valid tokens
    m3 = mask.reshape(B, KT, P)
    v3 = valid.reshape(B, KT, P)
    lo = np.where(v3, m3, np.iinfo(np.int64).max).min(axis=2)  # [B, KT]
    hi = np.where(v3, m3, -1).max(axis=2)                      # [B, KT]

    klists = []
    for r in range(RPC):
        rows = [c * RPC + r for c in range(NCORES)]
        per_m = []
        for m in range(MT):
            ks = [k for k in range(KT)
                  if any(lo[b, k] <= m * P + P - 1 and hi[b, k] >= m * P
                         for b in rows)]
            per_m.append(ks if ks else [0])
        klists.append(per_m)

    counts = np.zeros((B, NSEG), np.int64)
    for b in range(B):
        ids = mask[b][valid[b]]
        ids = ids[ids < NSEG]
        counts[b] = np.bincount(ids, minlength=NSEG)
    invc = (1.0 / np.maximum(counts, 1)).astype(np.float32)

    maskp = np.ascontiguousarray(
        mask.astype(np.float32).reshape(B, KT, P).transpose(0, 2, 1))
    invcp = np.ascontiguousarray(
        invc.reshape(B, MT, P).transpose(0, 2, 1))

    in_maps = [{
        "x": hs[c * RPC:(c + 1) * RPC],
        "maskp": maskp[c * RPC:(c + 1) * RPC],
        "invc": invcp[c * RPC:(c + 1) * RPC],
    } for c in range(NCORES)]
    return klists, in_maps


_PROGRAM_CACHE = {}


def _get_program(klists):
    key = tuple(tuple(tuple(ks) for ks in per_m) for per_m in klists)
    if key not in _PROGRAM_CACHE:
        _PROGRAM_CACHE[key] = _build_program(klists)
    return _PROGRAM_CACHE[key]


def kernel(hidden_states, output_mask):
    klists, in_maps = _prep(hidden_states, output_mask)
    nc = _get_program(klists)
    res = run_bass_kernel_spmd(nc, in_maps, core_ids=list(range(NCORES)))
    full = np.concatenate(
        [res.results[c]["out"].reshape(RPC * NSEG, H) for c in range(NCORES)],
        axis=0)
    return full


if __name__ == "__main__":
    rng = np.random.default_rng(0)
    hs = rng.standard_normal((B, S, H)).astype(np.float32)
    mask = np.sort(rng.integers(0, NSEG, size=(B, S)), axis=-1).astype(np.int32)
    out = kernel(hidden_states=hs, output_mask=mask)
    print(out.shape, out.dtype)


# revision 6
# speedup vs baseline: 780.8355x; 780.8355x over previous
"""Segment mean-pool (ContextEncoder) Bass kernel for Trainium2, 8 NeuronCores.

Problem: hidden_states [16, 4096, 1024] f32, output_mask [16, 4096] int
(non-decreasing segment ids per row in [0, 512), -1 = dropped token).
Output [16*512, 1024] f32: mean of tokens sharing (batch, segment id),
zeros for empty segments.

Strategy: data-parallel over batch, 2 rows per core. Per 128-token K-tile,
build a one-hot [tokens x 512 segments] matrix on the vector engine
(iota vs per-partition segment id, is_equal), then accumulate
one_hot.T @ x on the tensor engine (fp32r) into PSUM, one [128 seg x 1024]
region per 128-segment M-tile. Because ids are sorted, each K-tile only
touches 1-2 M-tiles; the (k -> M-tiles) map is computed on the host from
the actual masks (the program is compiled per input batch) so the matmul
count stays near the minimum while remaining exact for any mask content.
Mean = PSUM * (1/count) on the way out, with counts from a host bincount.
"""

import numpy as np

import concourse.bass as bass  # noqa: F401  (registers bass_rust)
import concourse.mybir as mybir
import concourse.tile as tile
from concourse import bacc
from concourse.bass_utils import run_bass_kernel_spmd

B, S, H = 16, 4096, 1024
NSEG = 512
NCORES = 8
RPC = B // NCORES          # rows (batch examples) per core
P = 128                    # partitions
KT = S // P                # 32 K-tiles of 128 tokens
MT = NSEG // P             # 4 M-tiles of 128 segments
NH = H // 512              # matmul free-dim chunks (PSUM bank = 512 f32)

F32 = mybir.dt.float32
F32R = mybir.dt.float32r   # full-rate fp32 matmul mode on TRN2

# Number of SBUF buffers for data tiles (DMA prefetch depth)
DATA_BUFS = 8
OH_BUFS = 6
OSB_BUFS = 4


def _build_program(klists, loop_n=1):
    """klists[r][m] -> sorted list of K-tile indices whose token ids (in any
    row assigned to program slot r) overlap segment M-tile m. Must be
    non-empty for every (r, m).

    loop_n > 1 wraps the body in an in-NEFF repeat loop (timing only)."""
    nc = bacc.Bacc("TRN2", target_bir_lowering=False, debug=False)
    x = nc.dram_tensor("x", [RPC, S, H], F32R, kind="ExternalInput")
    maskp = nc.dram_tensor("maskp", [RPC, P, KT], F32, kind="ExternalInput")
    invc = nc.dram_tensor("invc", [RPC, P, MT], F32, kind="ExternalInput")
    out = nc.dram_tensor("out", [RPC, NSEG, H], F32, kind="ExternalOutput")

    with tile.TileContext(nc) as tc:
        with tc.tile_pool(name="const", bufs=1) as cpool, \
             tc.tile_pool(name="data", bufs=DATA_BUFS) as dpool, \
             tc.tile_pool(name="oh", bufs=OH_BUFS) as opool, \
             tc.tile_pool(name="osb", bufs=OSB_BUFS) as spool, \
             tc.tile_pool(name="ps", bufs=MT, space="PSUM") as pspool:
            iota_t = cpool.tile([P, NSEG], F32, tag="iota")
            nc.gpsimd.iota(iota_t[:], [[1, NSEG]], channel_multiplier=0,
                           allow_small_or_imprecise_dtypes=True)
            body = _make_body(nc, klists, x, maskp, invc, out, iota_t,
                              cpool, dpool, opool, spool, pspool)
            if loop_n > 1:
                with tc.For_i(0, loop_n, 1):
                    body()
            else:
                body()
    nc.compile()
    return nc


def _make_body(nc, klists, x, maskp, invc, out, iota_t,
               cpool, dpool, opool, spool, pspool):
    def body():
        for r in range(RPC):
                mask_sb = cpool.tile([P, KT], F32, tag=f"mask{r}")
                nc.sync.dma_start(out=mask_sb[:], in_=maskp[r])
                invc_sb = cpool.tile([P, MT], F32, tag=f"invc{r}")
                nc.sync.dma_start(out=invc_sb[:], in_=invc[r])

                k_to_ms = {}
                for m in range(MT):
                    for k in klists[r][m]:
                        k_to_ms.setdefault(k, []).append(m)
                firsts = {m: klists[r][m][0] for m in range(MT)}
                lasts = {m: klists[r][m][-1] for m in range(MT)}

                psum = [pspool.tile([P, H], F32, tag="ps", name=f"psum_r{r}m{m}")
                        for m in range(MT)]

                for k in sorted(k_to_ms):
                    data_t = dpool.tile([P, H], F32R, tag="data")
                    nc.sync.dma_start(out=data_t[:], in_=x[r, k * P:(k + 1) * P, :])
                    oh = opool.tile([P, NSEG], F32R, tag="oh")
                    nc.vector.tensor_scalar(
                        out=oh[:], in0=iota_t[:], scalar1=mask_sb[:, k:k + 1],
                        scalar2=None, op0=mybir.AluOpType.is_equal)
                    for m in k_to_ms[k]:
                        for n in range(NH):
                            nc.tensor.matmul(
                                out=psum[m][:, n * 512:(n + 1) * 512],
                                lhsT=oh[:, m * P:(m + 1) * P],
                                rhs=data_t[:, n * 512:(n + 1) * 512],
                                start=(k == firsts[m]), stop=(k == lasts[m]))

                for m in range(MT):
                    osb = spool.tile([P, H], F32, tag="osb")
                    nc.vector.tensor_scalar_mul(osb[:], psum[m][:], invc_sb[:, m:m + 1])
                    nc.sync.dma_start(out=out[r, m * P:(m + 1) * P, :], in_=osb[:])
    return body


def _prep(hidden_states, output_mask):
    hs = np.ascontiguousarray(np.asarray(hidden_states, dtype=np.float32))
    assert hs.shape == (B, S, H), hs.shape
    mask = np.asarray(output_mask).astype(np.int64)
    assert mask.shape == (B, S), mask.shape

    valid = mask >= 0
    # per-(row, K-tile) id range over valid tokens
    m3 = mask.reshape(B, KT, P)
    v3 = valid.reshape(B, KT, P)
    lo = np.where(v3, m3, np.iinfo(np.int64).max).min(axis=2)  # [B, KT]
    hi = np.where(v3, m3, -1).max(axis=2)                      # [B, KT]

    klists = []
    for r in range(RPC):
        rows = [c * RPC + r for c in range(NCORES)]
        per_m = []
        for m in range(MT):
            ks = [k for k in range(KT)
                  if any(lo[b, k] <= m * P + P - 1 and hi[b, k] >= m * P
                         for b in rows)]
            per_m.append(ks if ks else [0])
        klists.append(per_m)

    counts = np.zeros((B, NSEG), np.int64)
    for b in range(B):
        ids = mask[b][valid[b]]
        ids = ids[ids < NSEG]
        counts[b] = np.bincount(ids, minlength=NSEG)
    invc = (1.0 / np.maximum(counts, 1)).astype(np.float32)

    maskp = np.ascontiguousarray(
        mask.astype(np.float32).reshape(B, KT, P).transpose(0, 2, 1))
    invcp = np.ascontiguousarray(
        invc.reshape(B, MT, P).transpose(0, 2, 1))

    in_maps = [{
        "x": hs[c * RPC:(c + 1) * RPC],
        "maskp": maskp[c * RPC:(c + 1) * RPC],
        "invc": invcp[c * RPC:(c + 1) * RPC],
    } for c in range(NCORES)]
    return klists, in_maps


_PROGRAM_CACHE = {}


def _get_program(klists):
    key = tuple(tuple(tuple(ks) for ks in per_m) for per_m in klists)
    if key not in _PROGRAM_CACHE:
        _PROGRAM_CACHE[key] = _build_program(klists)
    return _PROGRAM_CACHE[key]


def kernel(hidden_states, output_mask):
    klists, in_maps = _prep(hidden_states, output_mask)
    nc = _get_program(klists)
    res = run_bass_kernel_spmd(nc, in_maps, core_ids=list(range(NCORES)))
    full = np.concatenate(
        [res.results[c]["out"].reshape(RPC * NSEG, H) for c in range(NCORES)],
        axis=0)
    return full


if __name__ == "__main__":
    rng = np.random.default_rng(0)
    hs = rng.standard_normal((B, S, H)).astype(np.float32)
    mask = np.sort(rng.integers(0, NSEG, size=(B, S)), axis=-1).astype(np.int32)
    out = kernel(hidden_states=hs, output_mask=mask)
    print(out.shape, out.dtype)
